# revision 12
# baseline (speedup 1.0000x reference)
"""CASSI shear kernel for Trainium2 (Bass/Tile), 8-core SPMD.

Computes, for full inputs x (1, 1024, 1024, 31) and ca (1, 1024, 1024, 1):
    y1[m, n, l] = x[m, n, l] * ca[m, n]
    out[m, t]   = sum_{n+l=t} y1[m, n, l]       (t in [0, 1054))
returning (1, 1024, 1054, 1) float32.

Sharding: rows m across 8 cores (128 rows/core = one full SBUF partition
block). Per core, free dim holds the (n, l) plane contiguously (n-major,
matching HBM layout so DMA loads are fully contiguous per partition).

Design (v4, DMA-bound by construction):
  - DMA (sync/HWDGE ring): x streamed in progressive column segments --
    small head segment so the first multiply starts early, large middle
    segments for line-rate HBM, tiny tail segment so the final
    mul->shear->evac chain after the last DMA byte is short. ca/ident/out
    ride the scalar (ACT) HWDGE ring so they interleave with the x stream
    instead of queueing in front of it.
  - Vector engine: y1 = x * ca broadcast multiply written as bf16 in the
    same n-major layout (contiguous writes; fp32 reads are the 1x DVE
    path). bf16 keeps ~3 significant digits -- rel err ~4e-3 vs this
    problem's 2e-2 gate.
  - Tensor engine: the shear scatter-add as 64 identity-weight bf16
    matmuls accumulating into PSUM, one per 16-wide n-window: rhs
    element (j, k) reads y1[t0+j, k] -- the k-runs are CONTIGUOUS in
    SBUF (l is the fastest axis), which keeps the PE moving-operand
    fetch on its fast path -- and dst element (j, k) writes PSUM column
    t0+j+k (same-address revisits 30 rows apart; PSUM accumulation is
    an in-memory fp32 add). 16*31 = 496 rows <= the 512-element ISA
    cap. Each window reads one 16-column slice of y1 exactly once, so
    no edge cases and no guard columns exist. A window whose 46-wide
    dst span would cross a PSUM bank boundary accumulates instead into
    a second accumulator whose bank phase is shifted 256 columns (a
    46-wide span never crosses both grids); the evacuation folds it
    back. PSUM zeroing = one start=True zero-weight bf16 matmul per
    used bank of each accumulator.
  - Scalar engine: evacuates each PSUM bank to SBUF as soon as its last
    shear matmul retires (bank 0 mid-stream, the rest at the end), the
    vector engine folds the few phase-shifted columns on top, and the
    rows DMA out -- so only a tiny evac rides the critical tail.
"""

import sys

import numpy as np

if "/opt/trn_rl_repo" not in sys.path:
    sys.path.insert(0, "/opt/trn_rl_repo")

M, N, L = 1024, 1024, 31
ONC = N + L - 1  # 1054
NCORES = 8
R = M // NCORES  # 128 rows per core
BANK = 512  # PSUM bank size in fp32 elements
SEG_WIDTHS = (32, 192, 256, 256, 192, 80, 16)  # head small, mid big, tail tiny
WARMUP_MMS = 9  # dummy matmuls that ramp the PE clock during the DMA head
WIN = 16  # n-columns per shear matmul (WIN * L = 496 <= 512 ISA cap)
SPAN = WIN + L - 1  # dst columns touched per window (46)
SHIFT = 256  # bank-phase shift of the second accumulator
P2W = ONC + SHIFT  # second accumulator width (phys cols 256..256+ONC)

_cached_nc = {}


def _segments(widths):
    segs, n0 = [], 0
    for w in widths:
        segs.append((n0, w))
        n0 += w
    assert n0 == N, widths
    return segs


def _crosses(a, b, phase):
    """[a, b) crosses a bank boundary of the grid offset by `phase`."""
    return (a + phase) // BANK != (b - 1 + phase) // BANK


def _shear_pieces(segs):
    """Shear matmuls as per-segment lists of (t0, acc2, stop1, stop2).

    One matmul per 16-wide n-window [t0, t0+16): rhs rows (j, k) read
    y1[t0+j, k] (l fastest -> contiguous), dst writes PSUM column
    t0+j+k. acc2=True routes the matmul to the phase-shifted second
    accumulator (dst span [t0, t0+46) would cross a bank boundary of
    accumulator 1). stop1/stop2 mark the last matmul per bank of each
    accumulator. Also returns fold_after: segment -> list of
    (col_a, col_b, fold) evacuation pieces, where fold=True means the
    second accumulator must be added on top."""
    pieces = []  # [si, t0, acc2, stop1, stop2]

    def seg_of(n):
        for si, (n0, w) in enumerate(segs):
            if n0 <= n < n0 + w:
                return si
        raise AssertionError(n)

    for t0 in range(0, N, WIN):
        acc2 = _crosses(t0, t0 + SPAN, 0)
        if acc2:
            assert not _crosses(t0, t0 + SPAN, SHIFT), t0
        pieces.append([seg_of(t0), t0, acc2, False, False])

    # stop flags: last piece per (accumulator, bank)
    last1, last2 = {}, {}
    for idx, (si, t0, acc2, _, _) in enumerate(pieces):
        lo, hi = t0 // BANK, (t0 + SPAN - 1) // BANK
        if acc2:
            for b in range((t0 + SHIFT) // BANK, (t0 + SPAN - 1 + SHIFT) // BANK + 1):
                last2[b] = idx
        else:
            for b in range(lo, hi + 1):
                last1[b] = idx
    for idx in last1.values():
        pieces[idx][3] = True
    for idx in last2.values():
        pieces[idx][4] = True

    # fold ranges: union of acc2 pieces' dst spans (clipped to banks)
    fold_cols = set()
    for si, t0, acc2, _, _ in pieces:
        if acc2:
            fold_cols.update(range(t0, min(t0 + SPAN, ONC)))

    by_seg = {}
    for si, t0, acc2, stop1, stop2 in pieces:
        by_seg.setdefault(si, []).append((t0, acc2, stop1, stop2))

    # evacuation plan: bank 0 completes once every piece with dst in
    # [0, 512) has run; with WIN|all seg boundaries that is the segment
    # holding t0 = 512 - SPAN rounded down to WIN. Remaining banks
    # complete in the last segment.
    def seg_done(col_end):
        # last window whose span reaches below col_end
        t0_last = ((col_end - 1) // WIN) * WIN
        t0_last = min(t0_last, N - WIN)
        return seg_of(t0_last)

    evac_after = {}
    ranges = [(0, BANK), (BANK, ONC)]
    for a, b in ranges:
        si = seg_done(b)
        # split [a, b) into maximal runs of fold / no-fold columns
        subs = []
        c = a
        while c < b:
            f = c in fold_cols
            e = c
            while e < b and ((e in fold_cols) == f):
                e += 1
            subs.append((c, e, f))
            c = e
        evac_after.setdefault(si, []).append((a, b, subs))
    return by_seg, evac_after


def _build_nc(loop_iters=None, variant="full"):
    """Build the per-core Bass program. loop_iters wraps the body in an
    on-device For_i repeating the computation (for benchmarking); None
    runs it once. variant: "full", or "+"-joined flags out of
    {dma, mul, pe, evac}."""
    key = (loop_iters, variant)
    if key in _cached_nc:
        return _cached_nc[key]

    import concourse.bass as bass
    import concourse.mybir as mybir
    from concourse import bacc
    from concourse.tile import TileContext

    f32 = mybir.dt.float32
    bf16 = mybir.dt.bfloat16
    nc = bacc.Bacc("TRN2")

    xin = nc.dram_tensor("x", (R, N * L), f32, kind="ExternalInput")
    cain = nc.dram_tensor("ca", (R, N), f32, kind="ExternalInput")
    identin = nc.dram_tensor("ident", (R, R), bf16, kind="ExternalInput")
    outd = nc.dram_tensor("out", (R, ONC), f32, kind="ExternalOutput")

    if variant == "full":
        flags = {"dma", "mul", "pe", "evac"}
    else:
        flags = set(variant.split("+"))
    segs = _segments(SEG_WIDTHS)
    by_seg, evac_after = _shear_pieces(segs)
    maxw = max(SEG_WIDTHS)

    with TileContext(nc) as tc:
        with (
            tc.tile_pool(name="xp", bufs=3) as xp,
            tc.tile_pool(name="cp", bufs=1) as cp,
            tc.tile_pool(name="accp", bufs=1) as accp,
            tc.tile_pool(name="pp", bufs=1, space="PSUM") as pp,
        ):
            ca_t = cp.tile([R, N], f32)
            nc.scalar.dma_start(out=ca_t[:], in_=cain[:])
            idb = cp.tile([R, R], bf16, tag="idb")
            nc.scalar.dma_start(out=idb[:], in_=identin[:])
            zrhs = cp.tile([R, BANK], bf16, tag="zrhs")
            nc.gpsimd.memset(zrhs[:], 0.0)

            yt = cp.tile([R, N * L], bf16, tag="yt")

            acc = accp.tile([R, ONC], f32)
            pacc = pp.tile([R, ONC], f32, tag="p1")
            pacc2 = pp.tile([R, P2W], f32, tag="p2")
            pwarm = pp.tile([R, BANK], f32, tag="pwarm")

            # "touch" ca on the vector engine so the first segment's multiply
            # needs only one sync wait (TensorTensor has a single wait slot;
            # Bacc would otherwise spill onto an EventSemaphore nop)
            scr1 = cp.tile([R, 1], f32, tag="scr1")
            nc.vector.tensor_copy(scr1[:], ca_t[:, 0:1])

            yfull = yt[:]
            ypart = [int(yfull.ap[0][0]), int(yfull.ap[0][1])]

            def zero_banks(tile, lo, hi):
                for a in range(lo, hi, BANK):
                    b = min(a + BANK, hi)
                    nc.tensor.matmul(
                        tile[:, a:b],
                        zrhs[:, 0:R],
                        zrhs[:, 0 : b - a],
                        start=True, stop=False, skip_group_check=True,
                    )

            def body():
                # Reset PSUM has_written bits and zero the accumulators: one
                # start=True zero-weight bf16 matmul per used bank.
                if "pe" in flags:
                    zero_banks(pacc, 0, ONC)
                    zero_banks(pacc2, BANK, P2W)
                    # ramp the PE clock (HAM needs ~3.4us of sustained busy)
                    # while the first x segments stream in; the results are
                    # never read
                    for _ in range(WARMUP_MMS):
                        nc.tensor.matmul(
                            pwarm[:],
                            zrhs[:, 0:R],
                            zrhs[:, 0:BANK],
                            start=True, stop=True, skip_group_check=True,
                        )
                for si, (n0, wseg) in enumerate(segs):
                    xt = xp.tile([R, maxw * L], f32, tag="xseg")
                    xv = xt[:, 0 : wseg * L]
                    if "dma" in flags:
                        nc.sync.dma_start(
                            out=xv, in_=xin[:, n0 * L : (n0 + wseg) * L]
                        )
                    if "mul" in flags:
                        x3 = xv.rearrange("p (n l) -> p n l", l=L)
                        cab = (
                            ca_t[:, n0 : n0 + wseg]
                            .unsqueeze(2)
                            .broadcast_to([R, wseg, L])
                        )
                        y3 = (
                            yfull[:, n0 * L : (n0 + wseg) * L]
                            .rearrange("p (n l) -> p n l", l=L)
                        )
                        nc.vector.tensor_tensor(
                            y3, x3, cab, mybir.AluOpType.mult
                        )
                    if "pe" in flags:
                        for t0, acc2, stop1, stop2 in by_seg.get(si, ()):
                            # rhs rows (j, k) = y1[t0+j, k]: k (the l axis)
                            # is innermost and contiguous. dst column
                            # t0+j+k: distinct within a k-run; the same
                            # column recurs 30 rows later (PSUM in-memory
                            # accumulate handles both).
                            rhs = bass.AP(
                                yfull.tensor,
                                yfull.offset + t0 * L,
                                [ypart, [L, WIN], [1, L]],
                            )
                            if acc2:
                                pv = pacc2[:, SHIFT + t0 : SHIFT + t0 + SPAN]
                            else:
                                pv = pacc[:, t0 : t0 + SPAN]
                            pp0 = [int(pv.ap[0][0]), int(pv.ap[0][1])]
                            dst = bass.AP(
                                pv.tensor, pv.offset, [pp0, [1, WIN], [1, L]]
                            )
                            nc.tensor.matmul(
                                dst, idb[:], rhs,
                                start=False,
                                stop=(stop2 if acc2 else stop1),
                                skip_group_check=True,
                            )
                    if "evac" in flags:
                        for a, b, subs in evac_after.get(si, ()):
                            nc.scalar.copy(acc[:, a:b], pacc[:, a:b])
                            for c, e, fold in subs:
                                if fold:
                                    nc.vector.tensor_tensor(
                                        acc[:, c:e],
                                        acc[:, c:e],
                                        pacc2[:, SHIFT + c : SHIFT + e],
                                        mybir.AluOpType.add,
                                    )
                            nc.scalar.dma_start(
                                out=outd[:, a:b], in_=acc[:, a:b]
                            )

            if loop_iters is None:
                body()
            else:
                with tc.For_i(0, loop_iters, 1):
                    body()

    nc.finalize()
    _cached_nc[key] = nc
    return nc


_IDENT = None


def _run(x_slab, ca_slab, loop_iters=None, variant="full", **run_kwargs):
    """x_slab (M, N*L) f32, ca_slab (M, N) f32 -> (M, ONC) f32."""
    from concourse.bass_utils import run_bass_kernel_spmd

    import ml_dtypes

    global _IDENT
    if _IDENT is None:
        _IDENT = np.eye(R, dtype=ml_dtypes.bfloat16)

    nc = _build_nc(loop_iters, variant)
    in_maps = []
    for c in range(NCORES):
        in_maps.append(
            {
                "x": np.ascontiguousarray(x_slab[c * R : (c + 1) * R]),
                "ca": np.ascontiguousarray(ca_slab[c * R : (c + 1) * R]),
                "ident": _IDENT,
            }
        )
    res = run_bass_kernel_spmd(nc, in_maps, core_ids=list(range(NCORES)), **run_kwargs)
    out = np.concatenate(
        [np.asarray(res.results[c]["out"]) for c in range(NCORES)], axis=0
    )
    return out, res


def kernel(x, ca):
    x = np.ascontiguousarray(np.asarray(x, dtype=np.float32).reshape(M, N * L))
    ca = np.ascontiguousarray(np.asarray(ca, dtype=np.float32).reshape(M, N))
    out, _ = _run(x, ca)
    return out.reshape(1, M, ONC, 1)


# revision 14
# speedup vs baseline: 1.5953x; 1.5953x over previous
"""CASSI shear kernel for Trainium2 (Bass/Tile), 8-core SPMD.

Computes, for full inputs x (1, 1024, 1024, 31) and ca (1, 1024, 1024, 1):
    y1[m, n, l] = x[m, n, l] * ca[m, n]
    out[m, t]   = sum_{n+l=t} y1[m, n, l]       (t in [0, 1054))
returning (1, 1024, 1054, 1) float32.

Sharding: rows m across 8 cores (128 rows/core = one full SBUF partition
block). Per core, free dim holds the (n, l) plane contiguously (n-major,
matching HBM layout so DMA loads are fully contiguous per partition).

Design (v4, DMA-bound by construction):
  - DMA (sync/HWDGE ring): x streamed in progressive column segments --
    small head segment so the first multiply starts early, large middle
    segments for line-rate HBM, tiny tail segment so the final
    mul->shear->evac chain after the last DMA byte is short. ca/ident/out
    ride the scalar (ACT) HWDGE ring so they interleave with the x stream
    instead of queueing in front of it.
  - Vector engine: y1 = x * ca broadcast multiply written as bf16 in the
    same n-major layout (contiguous writes; fp32 reads are the 1x DVE
    path). bf16 keeps ~3 significant digits -- rel err ~4e-3 vs this
    problem's 2e-2 gate.
  - Tensor engine: the shear scatter-add as 64 identity-weight bf16
    matmuls accumulating into PSUM, one per 16-wide n-window: rhs
    element (j, k) reads y1[t0+j, k] -- the k-runs are CONTIGUOUS in
    SBUF (l is the fastest axis), which keeps the PE moving-operand
    fetch on its fast path -- and dst element (j, k) writes PSUM column
    t0+j+k (same-address revisits 30 rows apart; PSUM accumulation is
    an in-memory fp32 add). 16*31 = 496 rows <= the 512-element ISA
    cap. Each window reads one 16-column slice of y1 exactly once, so
    no edge cases and no guard columns exist. A window whose 46-wide
    dst span would cross a PSUM bank boundary accumulates instead into
    a second accumulator whose bank phase is shifted 256 columns (a
    46-wide span never crosses both grids); the evacuation folds it
    back. PSUM zeroing = one start=True zero-weight bf16 matmul per
    used bank of each accumulator.
  - Scalar engine: evacuates each PSUM bank to SBUF as soon as its last
    shear matmul retires (bank 0 mid-stream, the rest at the end), the
    vector engine folds the few phase-shifted columns on top, and the
    rows DMA out -- so only a tiny evac rides the critical tail.
"""

import sys

import numpy as np

if "/opt/trn_rl_repo" not in sys.path:
    sys.path.insert(0, "/opt/trn_rl_repo")

M, N, L = 1024, 1024, 31
ONC = N + L - 1  # 1054
NCORES = 8
R = M // NCORES  # 128 rows per core
BANK = 512  # PSUM bank size in fp32 elements
SEG_WIDTHS = (32, 192, 256, 256, 192, 80, 16)  # head small, mid big, tail tiny
WARMUP_MMS = 9  # dummy matmuls that ramp the PE clock during the DMA head
WIN = 16  # n-columns per shear matmul (WIN * L = 496 <= 512 ISA cap)
SPAN = WIN + L - 1  # dst columns touched per window (46)
SHIFT = 256  # bank-phase shift of the second accumulator
P2W = ONC + SHIFT  # second accumulator width (phys cols 256..256+ONC)

_cached_nc = {}


def _segments(widths):
    segs, n0 = [], 0
    for w in widths:
        segs.append((n0, w))
        n0 += w
    assert n0 == N, widths
    return segs


def _crosses(a, b, phase):
    """[a, b) crosses a bank boundary of the grid offset by `phase`."""
    return (a + phase) // BANK != (b - 1 + phase) // BANK


def _shear_pieces(segs):
    """Shear matmuls as per-segment lists of (t0, acc2, stop1, stop2).

    One matmul per 16-wide n-window [t0, t0+16): rhs rows (j, k) read
    y1[t0+j, k] (l fastest -> contiguous), dst writes PSUM column
    t0+j+k. acc2=True routes the matmul to the phase-shifted second
    accumulator (dst span [t0, t0+46) would cross a bank boundary of
    accumulator 1). stop1/stop2 mark the last matmul per bank of each
    accumulator. Also returns fold_after: segment -> list of
    (col_a, col_b, fold) evacuation pieces, where fold=True means the
    second accumulator must be added on top."""
    pieces = []  # [si, t0, acc2, stop1, stop2]

    def seg_of(n):
        for si, (n0, w) in enumerate(segs):
            if n0 <= n < n0 + w:
                return si
        raise AssertionError(n)

    for t0 in range(0, N, WIN):
        acc2 = _crosses(t0, t0 + SPAN, 0)
        if acc2:
            assert not _crosses(t0, t0 + SPAN, SHIFT), t0
        pieces.append([seg_of(t0), t0, acc2, False, False])

    # stop flags: last piece per (accumulator, bank)
    last1, last2 = {}, {}
    for idx, (si, t0, acc2, _, _) in enumerate(pieces):
        lo, hi = t0 // BANK, (t0 + SPAN - 1) // BANK
        if acc2:
            for b in range((t0 + SHIFT) // BANK, (t0 + SPAN - 1 + SHIFT) // BANK + 1):
                last2[b] = idx
        else:
            for b in range(lo, hi + 1):
                last1[b] = idx
    for idx in last1.values():
        pieces[idx][3] = True
    for idx in last2.values():
        pieces[idx][4] = True

    # fold ranges: union of acc2 pieces' dst spans (clipped to banks)
    fold_cols = set()
    for si, t0, acc2, _, _ in pieces:
        if acc2:
            fold_cols.update(range(t0, min(t0 + SPAN, ONC)))

    by_seg = {}
    for si, t0, acc2, stop1, stop2 in pieces:
        by_seg.setdefault(si, []).append((t0, acc2, stop1, stop2))

    # evacuation plan: split [0, ONC) into atoms at fold-run edges,
    # compute for each atom the segment in which its last writer runs,
    # merge adjacent atoms completing in the same segment, and emit each
    # range right after that segment's matmuls. Only the final range
    # rides the critical tail.
    def last_writer_seg(a, b):
        best = 0
        for si, t0, acc2, _, _ in pieces:
            if t0 < b and t0 + SPAN > a:
                best = max(best, si)
        return best

    edges = {0, ONC}
    c = 0
    while c < ONC:
        f = c in fold_cols
        e = c
        while e < ONC and ((e in fold_cols) == f):
            e += 1
        edges.add(e)
        c = e
    edges = sorted(edges)
    atoms = [
        (a, b, last_writer_seg(a, b), (a in fold_cols))
        for a, b in zip(edges, edges[1:])
    ]
    merged = []
    for a, b, si, f in atoms:
        if merged and merged[-1][2] == si:
            ma, _, _, msubs = merged[-1]
            msubs.append((a, b, f))
            merged[-1] = (ma, b, si, msubs)
        else:
            merged.append((a, b, si, [(a, b, f)]))
    evac_after = {}
    for a, b, si, subs in merged:
        evac_after.setdefault(si, []).append((a, b, subs))
    return by_seg, evac_after


def _build_nc(loop_iters=None, variant="full"):
    """Build the per-core Bass program. loop_iters wraps the body in an
    on-device For_i repeating the computation (for benchmarking); None
    runs it once. variant: "full", or "+"-joined flags out of
    {dma, mul, pe, evac}."""
    key = (loop_iters, variant)
    if key in _cached_nc:
        return _cached_nc[key]

    import concourse.bass as bass
    import concourse.mybir as mybir
    from concourse import bacc
    from concourse.tile import TileContext

    f32 = mybir.dt.float32
    bf16 = mybir.dt.bfloat16
    nc = bacc.Bacc("TRN2")

    xin = nc.dram_tensor("x", (R, N * L), f32, kind="ExternalInput")
    cain = nc.dram_tensor("ca", (R, N), f32, kind="ExternalInput")
    identin = nc.dram_tensor("ident", (R, R), bf16, kind="ExternalInput")
    outd = nc.dram_tensor("out", (R, ONC), f32, kind="ExternalOutput")

    if variant == "full":
        flags = {"dma", "mul", "pe", "evac"}
    else:
        flags = set(variant.split("+"))
    segs = _segments(SEG_WIDTHS)
    by_seg, evac_after = _shear_pieces(segs)
    maxw = max(SEG_WIDTHS)

    with TileContext(nc) as tc:
        with (
            tc.tile_pool(name="xp", bufs=3) as xp,
            tc.tile_pool(name="cp", bufs=1) as cp,
            tc.tile_pool(name="accp", bufs=1) as accp,
            tc.tile_pool(name="pp", bufs=1, space="PSUM") as pp,
        ):
            ca_t = cp.tile([R, N], f32)
            nc.scalar.dma_start(out=ca_t[:], in_=cain[:])
            idb = cp.tile([R, R], bf16, tag="idb")
            nc.scalar.dma_start(out=idb[:], in_=identin[:])
            zrhs = cp.tile([R, BANK], bf16, tag="zrhs")
            nc.gpsimd.memset(zrhs[:], 0.0)

            yt = cp.tile([R, N * L], bf16, tag="yt")

            acc = accp.tile([R, ONC], f32)
            pacc = pp.tile([R, ONC], f32, tag="p1")
            pacc2 = pp.tile([R, P2W], f32, tag="p2")
            pwarm = pp.tile([R, BANK], f32, tag="pwarm")

            # "touch" ca on the vector engine so the first segment's multiply
            # needs only one sync wait (TensorTensor has a single wait slot;
            # Bacc would otherwise spill onto an EventSemaphore nop)
            scr1 = cp.tile([R, 1], f32, tag="scr1")
            nc.vector.tensor_copy(scr1[:], ca_t[:, 0:1])

            yfull = yt[:]
            ypart = [int(yfull.ap[0][0]), int(yfull.ap[0][1])]

            def zero_banks(tile, lo, hi):
                for a in range(lo, hi, BANK):
                    b = min(a + BANK, hi)
                    nc.tensor.matmul(
                        tile[:, a:b],
                        zrhs[:, 0:R],
                        zrhs[:, 0 : b - a],
                        start=True, stop=False, skip_group_check=True,
                    )

            def body():
                # Reset PSUM has_written bits and zero the accumulators: one
                # start=True zero-weight bf16 matmul per used bank.
                if "pe" in flags:
                    zero_banks(pacc, 0, ONC)
                    zero_banks(pacc2, BANK, P2W)
                    # ramp the PE clock (HAM needs ~3.4us of sustained busy)
                    # while the first x segments stream in; the results are
                    # never read
                    for _ in range(WARMUP_MMS):
                        nc.tensor.matmul(
                            pwarm[:],
                            zrhs[:, 0:R],
                            zrhs[:, 0:BANK],
                            start=True, stop=True, skip_group_check=True,
                        )
                for si, (n0, wseg) in enumerate(segs):
                    xt = xp.tile([R, maxw * L], f32, tag="xseg")
                    xv = xt[:, 0 : wseg * L]
                    if "dma" in flags:
                        nc.sync.dma_start(
                            out=xv, in_=xin[:, n0 * L : (n0 + wseg) * L]
                        )
                    if "mul" in flags:
                        x3 = xv.rearrange("p (n l) -> p n l", l=L)
                        cab = (
                            ca_t[:, n0 : n0 + wseg]
                            .unsqueeze(2)
                            .broadcast_to([R, wseg, L])
                        )
                        y3 = (
                            yfull[:, n0 * L : (n0 + wseg) * L]
                            .rearrange("p (n l) -> p n l", l=L)
                        )
                        nc.vector.tensor_tensor(
                            y3, x3, cab, mybir.AluOpType.mult
                        )
                    if "pe" in flags:
                        for t0, acc2, stop1, stop2 in by_seg.get(si, ()):
                            # rhs rows (j, k) = y1[t0+j, k]: k (the l axis)
                            # is innermost and contiguous. dst column
                            # t0+j+k: distinct within a k-run; the same
                            # column recurs 30 rows later (PSUM in-memory
                            # accumulate handles both).
                            rhs = bass.AP(
                                yfull.tensor,
                                yfull.offset + t0 * L,
                                [ypart, [L, WIN], [1, L]],
                            )
                            if acc2:
                                pv = pacc2[:, SHIFT + t0 : SHIFT + t0 + SPAN]
                            else:
                                pv = pacc[:, t0 : t0 + SPAN]
                            pp0 = [int(pv.ap[0][0]), int(pv.ap[0][1])]
                            dst = bass.AP(
                                pv.tensor, pv.offset, [pp0, [1, WIN], [1, L]]
                            )
                            nc.tensor.matmul(
                                dst, idb[:], rhs,
                                start=False,
                                stop=(stop2 if acc2 else stop1),
                                skip_group_check=True,
                            )
                    if "evac" in flags:
                        for a, b, subs in evac_after.get(si, ()):
                            # the last range rides the critical tail: keep it
                            # off the scalar engine (busy issuing the previous
                            # range's DMA) -- vector copies+folds it and the
                            # idle sync queue DMAs it out
                            last = si == len(segs) - 1
                            if last:
                                nc.vector.tensor_copy(
                                    acc[:, a:b], pacc[:, a:b]
                                )
                            else:
                                nc.scalar.copy(acc[:, a:b], pacc[:, a:b])
                            for c, e, fold in subs:
                                if fold:
                                    nc.vector.tensor_tensor(
                                        acc[:, c:e],
                                        acc[:, c:e],
                                        pacc2[:, SHIFT + c : SHIFT + e],
                                        mybir.AluOpType.add,
                                    )
                            dma_eng = nc.sync if last else nc.scalar
                            dma_eng.dma_start(
                                out=outd[:, a:b], in_=acc[:, a:b]
                            )

            if loop_iters is None:
                body()
            else:
                with tc.For_i(0, loop_iters, 1):
                    body()

    nc.finalize()
    _cached_nc[key] = nc
    return nc


_IDENT = None


def _run(x_slab, ca_slab, loop_iters=None, variant="full", **run_kwargs):
    """x_slab (M, N*L) f32, ca_slab (M, N) f32 -> (M, ONC) f32."""
    from concourse.bass_utils import run_bass_kernel_spmd

    import ml_dtypes

    global _IDENT
    if _IDENT is None:
        _IDENT = np.eye(R, dtype=ml_dtypes.bfloat16)

    nc = _build_nc(loop_iters, variant)
    in_maps = []
    for c in range(NCORES):
        in_maps.append(
            {
                "x": np.ascontiguousarray(x_slab[c * R : (c + 1) * R]),
                "ca": np.ascontiguousarray(ca_slab[c * R : (c + 1) * R]),
                "ident": _IDENT,
            }
        )
    res = run_bass_kernel_spmd(nc, in_maps, core_ids=list(range(NCORES)), **run_kwargs)
    out = np.concatenate(
        [np.asarray(res.results[c]["out"]) for c in range(NCORES)], axis=0
    )
    return out, res


def kernel(x, ca):
    x = np.ascontiguousarray(np.asarray(x, dtype=np.float32).reshape(M, N * L))
    ca = np.ascontiguousarray(np.asarray(ca, dtype=np.float32).reshape(M, N))
    out, _ = _run(x, ca)
    return out.reshape(1, M, ONC, 1)
